# revision 1
# baseline (speedup 1.0000x reference)
import math
import numpy as np

# Problem: nn_AKT_27917287424232 (sparse_attention)
# Shapes hardcoded per spec: B=8, S=1024, D=512, H=8.
# Strategy: data-parallel over batch B across the 8 NeuronCores (one batch
# element per core); weights replicated. Each core runs the full 4-pass
# attention stack for its batch element; outputs are gathered on host.

B, S, D, H = 8, 1024, 512, 8
DK = D // H
NEG = -1e32
LN_EPS = 1e-5


def _build_per_example(jnp, jax):
    mask_cur = jnp.tril(jnp.ones((S, S), jnp.float32), 0).astype(bool)
    mask_str = jnp.tril(jnp.ones((S, S), jnp.float32), -1).astype(bool)
    mask_win = (
        jnp.tril(jnp.ones((S, S), jnp.float32), -1)
        - jnp.tril(jnp.ones((S, S), jnp.float32), -20)
    ).astype(bool)
    x = jnp.arange(S, dtype=jnp.float32)
    pos = jnp.abs(x[None, :] - x[:, None])

    def _split(t):  # [S, D] -> [H, S, dk]
        return t.reshape(S, H, DK).transpose(1, 0, 2)

    def _merge(t):  # [H, S, dk] -> [S, D]
        return t.transpose(1, 0, 2).reshape(S, D)

    def _attn(q, k, v, mask, gamma, use_maxout_scale):
        s = jnp.einsum('hid,hjd->hij', q, k) / math.sqrt(DK)
        s = jnp.where(mask[None], s, NEG)
        p = jax.nn.softmax(s, axis=-1)
        dist = jnp.sqrt(
            jnp.clip((jnp.sum(p, -1, keepdims=True) - jnp.cumsum(p, -1)) * pos,
                     0.0, None))
        g = -jnp.abs(gamma)[:, None, None]
        s = s * jnp.clip(jnp.exp(dist * g), 1e-5, 1e5)
        s = jnp.where(mask[None], s, NEG)
        a = jnp.where(mask[None], jax.nn.softmax(s, axis=-1), 0.0)
        if use_maxout_scale:
            a = a * jnp.minimum(1.0 / jnp.max(a, -1, keepdims=True), 5.0)
        return jnp.einsum('hij,hjd->hid', a, v)

    def per_example(xq, xs, Wq, bq, Wqw, bqw, Wv, bv, Wo, bo, Wow, bow,
                    gammas, ln_w, ln_b, Wc, bc):
        def ln(t, i):
            mu = jnp.mean(t, -1, keepdims=True)
            var = jnp.mean(jnp.square(t - mu), -1, keepdims=True)
            return (t - mu) / jnp.sqrt(var + LN_EPS) * ln_w[i] + ln_b[i]

        def mha(i, tq, tk, tv, mask, mask_w, maxout):
            q_ = _split(tq @ Wq[i] + bq[i])
            k_ = _split(tk @ Wq[i] + bq[i])
            v_ = _split(tv @ Wv[i] + bv[i])
            out = _merge(_attn(q_, k_, v_, mask, gammas[i], maxout)) @ Wo[i] + bo[i]
            if not maxout:
                return out
            qw = _split(tq @ Wqw[i] + bqw[i])
            kw = _split(tk @ Wqw[i] + bqw[i])
            outw = _merge(_attn(qw, kw, v_, mask_w, gammas[i], False)) @ Wow + bow
            return out, outw

        hq = ln(xq + mha(0, xq, xq, xq, mask_cur, None, False), 0)
        hs = ln(xs + mha(1, xs, xs, xs, mask_cur, None, False), 1)
        o, ow = mha(2, hq, hq, hs, mask_str, mask_win, True)
        h = hq + o
        hw = h + ow
        out = jnp.concatenate([ln(h, 2), ln(hw, 2)], axis=-1) @ Wc + bc
        return out, hq, hs

    return per_example


_CACHE = {}


def _get_fn(kind):
    if kind in _CACHE:
        return _CACHE[kind]
    import jax
    import jax.numpy as jnp
    per_example = _build_per_example(jnp, jax)
    w_axes = (None,) * 15
    if kind == 'pmap':
        fn = jax.pmap(per_example, in_axes=(0, 0) + w_axes)
    else:
        cpus = jax.devices('cpu')
        fn = jax.jit(jax.vmap(per_example, in_axes=(0, 0) + w_axes),
                     device=cpus[0])
    _CACHE[kind] = fn
    return fn


def kernel(q_emb, s_emb, Wq, bq, Wqw, bqw, Wv, bv, Wo, bo, Wow, bow,
           gammas, ln_w, ln_b, Wc, bc, lens=None, **_unused):
    args = (np.asarray(q_emb, np.float32), np.asarray(s_emb, np.float32),
            np.asarray(Wq, np.float32), np.asarray(bq, np.float32),
            np.asarray(Wqw, np.float32), np.asarray(bqw, np.float32),
            np.asarray(Wv, np.float32), np.asarray(bv, np.float32),
            np.asarray(Wo, np.float32), np.asarray(bo, np.float32),
            np.asarray(Wow, np.float32), np.asarray(bow, np.float32),
            np.asarray(gammas, np.float32), np.asarray(ln_w, np.float32),
            np.asarray(ln_b, np.float32), np.asarray(Wc, np.float32),
            np.asarray(bc, np.float32))
    try:
        import jax
        if len(jax.devices()) >= 8 and q_emb.shape[0] == len(jax.local_devices()[:8]):
            fn = _get_fn('pmap')
            out, hq, hs = fn(*args)
        else:
            raise RuntimeError('need 8 devices for pmap path')
    except Exception:
        fn = _get_fn('cpu')
        out, hq, hs = fn(*args)
    return (np.asarray(out, np.float32), np.asarray(hq, np.float32),
            np.asarray(hs, np.float32))


# revision 2
# speedup vs baseline: 1.4244x; 1.4244x over previous
import math
import numpy as np

# Problem: nn_AKT_27917287424232 (sparse_attention)
# Shapes hardcoded per spec: B=8, S=1024, D=512, H=8.
# Strategy: data-parallel over batch B across the 8 NeuronCores (one batch
# element per core); weights replicated. Each core runs the full 4-pass
# attention stack for its batch element; outputs are gathered on host.

B, S, D, H = 8, 1024, 512, 8
DK = D // H
NEG = -1e32
LN_EPS = 1e-5


def _build_per_example(jnp, jax):
    mask_cur = jnp.tril(jnp.ones((S, S), jnp.float32), 0).astype(bool)
    mask_str = jnp.tril(jnp.ones((S, S), jnp.float32), -1).astype(bool)
    mask_win = (
        jnp.tril(jnp.ones((S, S), jnp.float32), -1)
        - jnp.tril(jnp.ones((S, S), jnp.float32), -20)
    ).astype(bool)
    x = jnp.arange(S, dtype=jnp.float32)
    pos = jnp.abs(x[None, :] - x[:, None])

    def _split(t):  # [S, D] -> [H, S, dk]
        return t.reshape(S, H, DK).transpose(1, 0, 2)

    def _merge(t):  # [H, S, dk] -> [S, D]
        return t.transpose(1, 0, 2).reshape(S, D)

    def _attn(q, k, v, mask, gamma, use_maxout_scale):
        s = jnp.einsum('hid,hjd->hij', q, k) / math.sqrt(DK)
        s = jnp.where(mask[None], s, NEG)
        p = jax.nn.softmax(s, axis=-1)
        dist = jnp.sqrt(
            jnp.clip((jnp.sum(p, -1, keepdims=True) - jnp.cumsum(p, -1)) * pos,
                     0.0, None))
        g = -jnp.abs(gamma)[:, None, None]
        s = s * jnp.clip(jnp.exp(dist * g), 1e-5, 1e5)
        s = jnp.where(mask[None], s, NEG)
        a = jnp.where(mask[None], jax.nn.softmax(s, axis=-1), 0.0)
        if use_maxout_scale:
            a = a * jnp.minimum(1.0 / jnp.max(a, -1, keepdims=True), 5.0)
        return jnp.einsum('hij,hjd->hid', a, v)

    def per_example(xq, xs, Wq, bq, Wqw, bqw, Wv, bv, Wo, bo, Wow, bow,
                    gammas, ln_w, ln_b, Wc, bc):
        def ln(t, i):
            mu = jnp.mean(t, -1, keepdims=True)
            var = jnp.mean(jnp.square(t - mu), -1, keepdims=True)
            return (t - mu) / jnp.sqrt(var + LN_EPS) * ln_w[i] + ln_b[i]

        def mha(i, tq, tk, tv, mask, mask_w, maxout):
            q_ = _split(tq @ Wq[i] + bq[i])
            k_ = _split(tk @ Wq[i] + bq[i])
            v_ = _split(tv @ Wv[i] + bv[i])
            out = _merge(_attn(q_, k_, v_, mask, gammas[i], maxout)) @ Wo[i] + bo[i]
            if not maxout:
                return out
            qw = _split(tq @ Wqw[i] + bqw[i])
            kw = _split(tk @ Wqw[i] + bqw[i])
            outw = _merge(_attn(qw, kw, v_, mask_w, gammas[i], False)) @ Wow + bow
            return out, outw

        hq = ln(xq + mha(0, xq, xq, xq, mask_cur, None, False), 0)
        hs = ln(xs + mha(1, xs, xs, xs, mask_cur, None, False), 1)
        o, ow = mha(2, hq, hq, hs, mask_str, mask_win, True)
        h = hq + o
        hw = h + ow
        out = jnp.concatenate([ln(h, 2), ln(hw, 2)], axis=-1) @ Wc + bc
        return out, hq, hs

    return per_example


_CACHE = {}


def _get_fn(kind):
    if kind in _CACHE:
        return _CACHE[kind]
    import jax
    import jax.numpy as jnp
    per_example = _build_per_example(jnp, jax)
    w_axes = (None,) * 15
    if kind == 'pmap':
        fn = jax.pmap(per_example, in_axes=(0, 0) + w_axes)
    else:
        cpus = jax.devices('cpu')
        fn = jax.jit(jax.vmap(per_example, in_axes=(0, 0) + w_axes),
                     device=cpus[0])
    _CACHE[kind] = fn
    return fn


def kernel(q_emb, s_emb, Wq, bq, Wqw, bqw, Wv, bv, Wo, bo, Wow, bow,
           gammas, ln_w, ln_b, Wc, bc, lens=None, **_unused):
    args = (np.asarray(q_emb, np.float32), np.asarray(s_emb, np.float32),
            np.asarray(Wq, np.float32), np.asarray(bq, np.float32),
            np.asarray(Wqw, np.float32), np.asarray(bqw, np.float32),
            np.asarray(Wv, np.float32), np.asarray(bv, np.float32),
            np.asarray(Wo, np.float32), np.asarray(bo, np.float32),
            np.asarray(Wow, np.float32), np.asarray(bow, np.float32),
            np.asarray(gammas, np.float32), np.asarray(ln_w, np.float32),
            np.asarray(ln_b, np.float32), np.asarray(Wc, np.float32),
            np.asarray(bc, np.float32))
    try:
        import jax
        if len(jax.devices()) < 8 or q_emb.shape[0] != 8:
            raise RuntimeError('need 8 devices for pmap path')
        fn = _get_fn('pmap')
        out, hq, hs = fn(*args)
        # materialize INSIDE the try: device failures can surface lazily
        res = (np.asarray(out, np.float32), np.asarray(hq, np.float32),
               np.asarray(hs, np.float32))
    except Exception:
        fn = _get_fn('cpu')
        out, hq, hs = fn(*args)
        res = (np.asarray(out, np.float32), np.asarray(hq, np.float32),
               np.asarray(hs, np.float32))
    return res
